# revision 1
# baseline (speedup 1.0000x reference)
# Bass/Tile kernel for nn_LongTermAttention (continuous long-term attention
# with rectangular basis functions) on 8 Trainium2 NeuronCores.
#
# Mathematical rewrite (verified exact vs the reference):
#   * G = F^T (F F^T + ridge I)^{-1} for the rectangular basis on the padded
#     uniform grid collapses to G[l, n] = (1/4.5) * [l // 4 == n], so
#     Bc[b,n,e] = (1/4.5) * sum_{j<4} k[b,e,4n+j]  (4-wide sum pooling).
#   * psi on the integration grid is a one-hot selector: for p < 999,
#     psi[n, p] = [n == floor(512 p / 999)]; column p=999 is all zero.
#     Hence scores[b,h,t,p] = s[b,h,t,n(p)] (piecewise constant) and the
#     P=1000-point continuous softmax reduces to basis space:
#       u_n   = exp(s_n + log Wn_n)          (Wn = quadrature mass per basis)
#       Z     = sum_n u_n + w_last           (w_last from the psi==0 column)
#       ctx   = (u / Z) @ values
#     The max-subtraction in the reference cancels exactly (scores are O(3),
#     exp is safe unstabilized).
#
# Sharding: data-parallel over batch, 2 batches per core; weights replicated.

import numpy as np

B_FULL = 16
N_CORES = 8
B_PER = B_FULL // N_CORES  # 2
E = 512          # embed dim
L = 2048         # memory length
T = 256          # query length
N = 512          # basis count
H = 8            # heads
D = 64           # head dim
P_GRID = 1000    # integration points
RIDGE_C = 4.5    # F F^T diag (4.0) + ridge (0.5)

_CACHE = {}


def _host_constants(Wk, Wv):
    """Fold pooling normalization (1/4.5) and query scale (1/8) into the
    projection weights; build log quadrature-mass vector."""
    import ml_dtypes
    wk = (Wk.astype(np.float64) / (RIDGE_C * 8.0)).astype(ml_dtypes.bfloat16)
    wv = (Wv.astype(np.float64) / RIDGE_C).astype(ml_dtypes.bfloat16)
    p = np.arange(P_GRID)
    nmap = (512 * p) // 999
    w = np.full(P_GRID, 1.0 / 999.0)
    w[0] = w[-1] = 1.0 / 1998.0
    Wn = np.zeros(N)
    for i in range(P_GRID - 1):
        Wn[nmap[i]] += w[i]
    lnw = np.ascontiguousarray(np.log(Wn).astype(np.float32).reshape(4, 128).T)
    w_last = float(w[-1])
    return wk, wv, lnw, w_last


def _build_program(w_last):
    import concourse.bass as bass
    import concourse.mybir as mybir
    import concourse.tile as tile
    from concourse import bacc
    from concourse.masks import make_identity

    f32 = mybir.dt.float32
    bf16 = mybir.dt.bfloat16

    nc = bacc.Bacc(
        "TRN2",
        target_bir_lowering=False,
        debug=False,
        enable_asserts=False,
        num_devices=N_CORES,
    )

    k_d = nc.dram_tensor("k", [B_PER, E, L], bf16, kind="ExternalInput").ap()
    q_d = nc.dram_tensor("q", [B_PER, T, E], bf16, kind="ExternalInput").ap()
    wk_d = nc.dram_tensor("wk", [E, E], bf16, kind="ExternalInput").ap()
    wv_d = nc.dram_tensor("wv", [E, E], bf16, kind="ExternalInput").ap()
    lnw_d = nc.dram_tensor("lnw", [128, 4], f32, kind="ExternalInput").ap()
    out_d = nc.dram_tensor("out", [B_PER, T, E], f32, kind="ExternalOutput").ap()

    from contextlib import ExitStack
    with tile.TileContext(nc) as tc, ExitStack() as ctx:
        _kernel_body(ctx, tc, nc, mybir, make_identity,
                     k_d, q_d, wk_d, wv_d, lnw_d, out_d, w_last)

    nc.compile()
    return nc


def _kernel_body(ctx, tc, nc, mybir, make_identity,
                 k_d, q_d, wk_d, wv_d, lnw_d, out_d, w_last):
    f32 = mybir.dt.float32
    bf16 = mybir.dt.bfloat16
    Exp = mybir.ActivationFunctionType.Exp

    def pool(name, bufs, space="SBUF"):
        return ctx.enter_context(tc.tile_pool(name=name, bufs=bufs, space=space))

    consts = pool("consts", 1)
    kpool = pool("kpool", 6)
    t1pool = pool("t1pool", 4)
    plpool = pool("plpool", 8)
    ktpool = pool("ktpool", 8)
    vpool = pool("vpool", 8)
    qpool = pool("qpool", 3)
    qtpool = pool("qtpool", 8)
    upool = pool("upool", 16)
    rzpool = pool("rzpool", 8)
    opool = pool("opool", 4)

    ps_proj = pool("ps_proj", 2, "PSUM")   # [128,512] tiles: 2 banks (also qT)
    ps_s = pool("ps_s", 2, "PSUM")         # [128,1024] tiles: 4 banks
    ps_c = pool("ps_c", 2, "PSUM")         # [128,65] tiles: 2 banks

    # ---- small constants first (they gate the projections / scores) ----
    wk_sb = consts.tile([128, 4 * 512], bf16, tag="wk")   # [e%128, (e//128)*512 + e']
    wv_sb = consts.tile([128, 4 * 512], bf16, tag="wv")
    nc.sync.dma_start(wk_sb[:].rearrange("p (kk e) -> p kk e", kk=4),
                      wk_d.rearrange("(kk p) e -> p kk e", p=128))
    nc.scalar.dma_start(wv_sb[:].rearrange("p (kk e) -> p kk e", kk=4),
                        wv_d.rearrange("(kk p) e -> p kk e", p=128))
    lnw_sb = consts.tile([128, 4], f32, tag="lnw")
    nc.sync.dma_start(lnw_sb[:], lnw_d[:])

    # ---- k DMA + pooling for BOTH batches (split over rings + engines),
    #      chunked at half-tiles so pooling starts as soon as data lands ----
    pooled_b = []
    for b in range(B_PER):
        pooled = []
        for et in range(4):
            pl = plpool.tile([128, N], bf16, tag="pl")
            eng = nc.vector if et % 2 == 0 else nc.gpsimd
            dma_eng = nc.sync if et % 2 == 0 else nc.scalar
            kt = kpool.tile([128, L], bf16, tag="k")
            dma_eng.dma_start(kt[:], k_d[b, et * 128:(et + 1) * 128, :])
            t1 = t1pool.tile([128, L // 2], f32,
                             tag="t1v" if et % 2 == 0 else "t1g")
            kv = kt[:].rearrange("p (i two) -> p i two", two=2)
            eng.tensor_add(t1[:], kv[:, :, 0], kv[:, :, 1])
            tv = t1[:].rearrange("p (i two) -> p i two", two=2)
            eng.tensor_add(pl[:], tv[:, :, 0], tv[:, :, 1])
            pooled.append(pl)
        pooled_b.append(pooled)

    # ---- qT via DMA xbar transpose on the scalar ring ----
    qT_b = []
    for b in range(B_PER):
        qT = []
        for eb in range(4):
            qt_sb = qtpool.tile([128, T], bf16, tag="qT")
            nc.sync.dma_start(qt_sb[:], q_d[b, :, eb * 128:(eb + 1) * 128],
                              transpose=True)
            qT.append(qt_sb)
        qT_b.append(qT)

    for b in range(B_PER):
        pooled = pooled_b[b]
        qT = qT_b[b]
        # ---- keysT = wk^T @ pooled  -> [e' (4x128 part), n=512] ----
        keysT = []
        for m in range(4):
            ps = ps_proj.tile([128, 512], f32, tag="ps_proj")
            for kk in range(4):
                nc.tensor.matmul(
                    ps[:],
                    wk_sb[:, kk * 512 + m * 128: kk * 512 + (m + 1) * 128],
                    pooled[kk][:],
                    start=(kk == 0), stop=(kk == 3),
                )
            kt_sb = ktpool.tile([128, 512], bf16, tag="keysT")
            nc.scalar.copy(kt_sb[:], ps[:])
            keysT.append(kt_sb)

        # ---- values = pooled^T @ wv -> [n (4x128 part), e'=512],
        #      stored head-interleaved with a ones column: [n, 8*(64+1)] ----
        values = []
        for m in range(4):
            ps = ps_proj.tile([128, 512], f32, tag="ps_proj")
            for kk in range(4):
                nc.tensor.matmul(
                    ps[:],
                    pooled[kk][:, m * 128:(m + 1) * 128],
                    wv_sb[:, kk * 512:(kk + 1) * 512],
                    start=(kk == 0), stop=(kk == 3),
                )
            v_sb = vpool.tile([128, 8 * 66], bf16, tag="values")
            vv = v_sb[:].rearrange("p (h c) -> p h c", c=66)
            nc.vector.tensor_copy(
                vv[:, :, 0:64],
                ps[:].rearrange("p (h d) -> p h d", d=64),
            )
            nc.vector.memset(vv[:, :, 64], 1.0)
            values.append(v_sb)

        # ---- per head pair: scores -> exp -> u -> ctx ----
        # Score pair goes to a 2-bank PSUM tile: head 2hp+h01 in bank h01
        # (cols h01*512 .. h01*512+256) so the two row-packed matmuls never
        # drain into the same bank concurrently.
        out_sbs = [opool.tile([128, E], f32, tag="out", name=f"out{b}_{mb}")
                   for mb in range(2)]
        for hp in range(4):
            if hp == 2:
                for mb in range(2):
                    nc.sync.dma_start(out_d[b, mb * 128:(mb + 1) * 128, 0:256],
                                      out_sbs[mb][:, 0:256])
            u_tiles = {}
            for nb in range(4):
                ps = ps_s.tile([128, 1024], f32, tag="ps_s")
                for h01 in range(2):
                    nc.tensor.matmul(
                        ps[:, h01 * 512: h01 * 512 + 256],
                        keysT[hp][h01 * 64:(h01 + 1) * 64,
                                  nb * 128:(nb + 1) * 128],
                        qT[hp][h01 * 64:(h01 + 1) * 64, :],
                        start=True, stop=True,
                        skip_group_check=True,
                    )
                u = upool.tile([128, 512], bf16, tag="u")
                nc.scalar.activation(
                    u[:].rearrange("p (g c) -> p g c", c=256),
                    ps[:].rearrange("p (g c) -> p g c", c=512)[:, :, 0:256],
                    Exp, bias=lnw_sb[:, nb:nb + 1], scale=1.0)
                u_tiles[nb] = u

            # ctx (+Z in col 64): one PSUM bank per (head, t-block) chain
            for h01 in range(2):
                h = hp * 2 + h01
                for mb in range(2):
                    ps = ps_c.tile([128, 65], f32, tag="ps_c")
                    for nb in range(4):
                        nc.tensor.matmul(
                            ps[:],
                            u_tiles[nb][:, h01 * 256 + mb * 128:
                                        h01 * 256 + (mb + 1) * 128],
                            values[nb][:, h * 66:h * 66 + 65],
                            start=(nb == 0), stop=(nb == 3),
                        )
                    rz = rzpool.tile([128, 1], f32, tag="rz")
                    nc.vector.tensor_scalar_add(rz[:], ps[:, 64:65], w_last)
                    rzi = rzpool.tile([128, 1], f32, tag="rzi")
                    nc.vector.reciprocal(rzi[:], rz[:])
                    nc.vector.tensor_scalar_mul(
                        out_sbs[mb][:, h * 64:(h + 1) * 64],
                        ps[:, 0:64], rzi[:])
        for mb in range(2):
            nc.sync.dma_start(out_d[b, mb * 128:(mb + 1) * 128, 256:512],
                              out_sbs[mb][:, 256:512])


def _get_program(w_last):
    if "nc" not in _CACHE:
        _CACHE["nc"] = _build_program(w_last)
    return _CACHE["nc"]


def make_in_maps(k, q, Wk, Wv):
    import ml_dtypes
    wk, wv, lnw, w_last = _host_constants(Wk, Wv)
    k16 = np.asarray(k).astype(ml_dtypes.bfloat16)
    q16 = np.asarray(q).astype(ml_dtypes.bfloat16)
    in_maps = []
    for c in range(N_CORES):
        in_maps.append({
            "k": np.ascontiguousarray(k16[c * B_PER:(c + 1) * B_PER]),
            "q": np.ascontiguousarray(q16[c * B_PER:(c + 1) * B_PER]),
            "wk": wk,
            "wv": wv,
            "lnw": lnw,
        })
    return in_maps, w_last


def kernel(k, q, Wk, Wv):
    from concourse.bass_utils import run_bass_kernel_spmd

    in_maps, w_last = make_in_maps(k, q, Wk, Wv)
    nc = _get_program(w_last)
    res = run_bass_kernel_spmd(nc, in_maps, core_ids=list(range(N_CORES)))
    return np.concatenate([res.results[c]["out"] for c in range(N_CORES)], axis=0)



# revision 3
# speedup vs baseline: 1.1805x; 1.1805x over previous
# Bass/Tile kernel for nn_LongTermAttention (continuous long-term attention
# with rectangular basis functions) on 8 Trainium2 NeuronCores.
#
# Mathematical rewrite (verified exact vs the reference):
#   * G = F^T (F F^T + ridge I)^{-1} for the rectangular basis on the padded
#     uniform grid collapses to G[l, n] = (1/4.5) * [l // 4 == n], so
#     Bc[b,n,e] = (1/4.5) * sum_{j<4} k[b,e,4n+j]  (4-wide sum pooling).
#   * psi on the integration grid is a one-hot selector, so the P=1000-point
#     continuous softmax reduces to basis space with per-basis quadrature
#     mass Wn:  p_n = exp(s_n) Wn_n / Z,  Z = sum_n exp(s_n) Wn_n + w_last,
#     ctx = p @ V.  Wn is folded into the values (V' = Wn V) and into the
#     Z-accumulator column, so the exp needs no bias at all.
#
# Layouts are prepared host-side (free):
#   * k is deinterleaved to [e, j, n] so the 4-wide pooling becomes two
#     unit-stride bf16 adds (DVE 2x mode) instead of stride-2 adds.
#   * q is pre-transposed to [e, t] so no on-chip transpose is needed.
#
# Sharding: data-parallel over batch, 2 batches per core; weights replicated.

import numpy as np

B_FULL = 16
N_CORES = 8
B_PER = B_FULL // N_CORES  # 2
E = 512          # embed dim
L = 2048         # memory length
T = 256          # query length
N = 512          # basis count
H = 8            # heads
D = 64           # head dim
P_GRID = 1000    # integration points
RIDGE_C = 4.5    # F F^T diag (4.0) + ridge (0.5)

_CACHE = {}


def _host_constants(Wk, Wv):
    """Fold pooling normalization (1/4.5) and query scale (1/8) into the
    projection weights; build the per-basis quadrature-mass column."""
    import ml_dtypes
    wk = (Wk.astype(np.float64) / (RIDGE_C * 8.0)).astype(ml_dtypes.bfloat16)
    wv = (Wv.astype(np.float64) / RIDGE_C).astype(ml_dtypes.bfloat16)
    p = np.arange(P_GRID)
    nmap = (512 * p) // 999
    w = np.full(P_GRID, 1.0 / 999.0)
    w[0] = w[-1] = 1.0 / 1998.0
    Wn = np.zeros(N)
    np.add.at(Wn, nmap[:-1], w[:-1])
    wn = np.ascontiguousarray(Wn.astype(np.float32).reshape(4, 128).T)  # [128,4]
    w_last = float(w[-1])
    return wk, wv, wn, w_last


def _build_program(w_last):
    import concourse.bass as bass
    import concourse.mybir as mybir
    import concourse.tile as tile
    from concourse import bacc

    f32 = mybir.dt.float32
    bf16 = mybir.dt.bfloat16

    nc = bacc.Bacc(
        "TRN2",
        target_bir_lowering=False,
        debug=False,
        enable_asserts=False,
        num_devices=N_CORES,
    )

    k_d = nc.dram_tensor("k", [B_PER, E, L], bf16, kind="ExternalInput").ap()
    qT_d = nc.dram_tensor("qT", [B_PER, E, T], bf16, kind="ExternalInput").ap()
    wk_d = nc.dram_tensor("wk", [E, E], bf16, kind="ExternalInput").ap()
    wv_d = nc.dram_tensor("wv", [E, E], bf16, kind="ExternalInput").ap()
    wn_d = nc.dram_tensor("wn", [128, 4], f32, kind="ExternalInput").ap()
    out_d = nc.dram_tensor("out", [B_PER, T, E], bf16, kind="ExternalOutput").ap()

    from contextlib import ExitStack
    with tile.TileContext(nc) as tc, ExitStack() as ctx:
        _kernel_body(ctx, tc, nc, mybir,
                     k_d, qT_d, wk_d, wv_d, wn_d, out_d, w_last)

    nc.compile()
    return nc


def _kernel_body(ctx, tc, nc, mybir,
                 k_d, qT_d, wk_d, wv_d, wn_d, out_d, w_last):
    f32 = mybir.dt.float32
    bf16 = mybir.dt.bfloat16
    Exp = mybir.ActivationFunctionType.Exp
    MULT = mybir.AluOpType.mult

    def pool(name, bufs, space="SBUF"):
        return ctx.enter_context(tc.tile_pool(name=name, bufs=bufs, space=space))

    consts = pool("consts", 1)
    kpool = pool("kpool", 5)
    t1pool = pool("t1pool", 3)
    plpool = pool("plpool", 8)
    qtpool = pool("qtpool", 2)
    ktpool = pool("ktpool", 8)
    vpool = pool("vpool", 8)
    upool = pool("upool", 12)
    rzpool = pool("rzpool", 4)
    opool = pool("opool", 4)

    ps_proj = pool("ps_proj", 2, "PSUM")   # [128,512] tiles: 1 bank each
    ps_s = pool("ps_s", 2, "PSUM")         # [128,1024] tiles: 2 banks each
    ps_c = pool("ps_c", 2, "PSUM")         # [128,260] tiles: 1 bank each

    # ---- constants on the gpsimd DMA ring (k saturates sync+scalar rings) ----
    wk_sb = consts.tile([128, 4 * 512], bf16, tag="wk")  # [e%128, kk*512+e']
    wv_sb = consts.tile([128, 4 * 512], bf16, tag="wv")
    wn_sb = consts.tile([128, 4], f32, tag="wn")
    nc.gpsimd.dma_start(wk_sb[:].rearrange("p (kk e) -> p kk e", kk=4),
                        wk_d.rearrange("(kk p) e -> p kk e", p=128))
    nc.gpsimd.dma_start(wv_sb[:].rearrange("p (kk e) -> p kk e", kk=4),
                        wv_d.rearrange("(kk p) e -> p kk e", p=128))
    nc.gpsimd.dma_start(wn_sb[:], wn_d[:])

    # ---- k DMA (both rings) + qT DMA; k cols are (j, n) deinterleaved ----
    kts = {}
    qt_b = []
    for b in range(B_PER):
        for et in range(4):
            ring = nc.sync if et % 2 == 0 else nc.scalar
            kt = kpool.tile([128, L], bf16, tag="k", name=f"kt{b}_{et}")
            ring.dma_start(kt[:], k_d[b, et * 128:(et + 1) * 128, :])
            kts[(b, et)] = kt
        qt = qtpool.tile([128, 4 * T], bf16, tag="qt", name=f"qt{b}")
        nc.sync.dma_start(qt[:].rearrange("p (eb t) -> p eb t", eb=4),
                          qT_d[b].rearrange("(eb p) t -> p eb t", p=128))
        qt_b.append(qt)

    # ---- pooling: two unit-stride bf16 adds per k tile (vector; et3->gpsimd) ----
    pooled_b = [[None] * 4 for _ in range(B_PER)]

    def emit_pool(b, et):
        kt = kts[(b, et)]
        eng = nc.gpsimd if et == 3 else nc.vector
        t1 = t1pool.tile([128, L // 2], bf16, tag="t1", name=f"t1_{b}_{et}")
        eng.tensor_add(t1[:], kt[:, 0:1024], kt[:, 1024:2048])
        pl = plpool.tile([128, N], bf16, tag="pl", name=f"pl{b}_{et}")
        eng.tensor_add(pl[:], t1[:, 0:512], t1[:, 512:1024])
        pooled_b[b][et] = pl

    for et in range(4):
        emit_pool(0, et)

    # ---- projections for a batch ----
    keysT_b = [[None] * 4 for _ in range(B_PER)]
    values_b = [[None] * 4 for _ in range(B_PER)]

    def emit_proj(b, kt_copy_eng):
        pooled = pooled_b[b]
        for m in range(4):
            # keysT[m] = wk^T @ pooled -> [e' (block m), n]
            ps = ps_proj.tile([128, 512], f32, tag="pp", name=f"psk{b}_{m}")
            for kk in range(4):
                nc.tensor.matmul(
                    ps[:],
                    wk_sb[:, kk * 512 + m * 128: kk * 512 + (m + 1) * 128],
                    pooled[kk][:],
                    start=(kk == 0), stop=(kk == 3),
                )
            kT = ktpool.tile([128, 512], bf16, tag="kT", name=f"kT{b}_{m}")
            if kt_copy_eng is nc.scalar:
                nc.scalar.copy(kT[:], ps[:])
            else:
                kt_copy_eng.tensor_copy(kT[:], ps[:])
            keysT_b[b][m] = kT

            # values[m] = pooled^T @ wv -> [n (block m), e'], scaled by Wn,
            # with the quadrature mass as a 65th column per head.
            ps2 = ps_proj.tile([128, 512], f32, tag="pp", name=f"psv{b}_{m}")
            for kk in range(4):
                nc.tensor.matmul(
                    ps2[:],
                    pooled[kk][:, m * 128:(m + 1) * 128],
                    wv_sb[:, kk * 512:(kk + 1) * 512],
                    start=(kk == 0), stop=(kk == 3),
                )
            v_sb = vpool.tile([128, 8 * 65], bf16, tag="v", name=f"v{b}_{m}")
            vv = v_sb[:].rearrange("p (h c) -> p h c", c=65)
            nc.vector.tensor_scalar_mul(
                vv[:, :, 0:64],
                ps2[:].rearrange("p (h d) -> p h d", d=64),
                wn_sb[:, m:m + 1])
            nc.vector.tensor_copy(vv[:, :, 64], wn_sb[:, m:m + 1].to_broadcast((128, 8)))
            values_b[b][m] = v_sb

    emit_proj(0, nc.scalar)

    for et in range(4):
        emit_pool(1, et)

    # ---- scores + exp for one head-pair hp: u[n, (h01, nbl, t)] tiles ----
    u_tiles = {}

    def emit_scores(b, hp):
        keysT = keysT_b[b]
        qt = qt_b[b]
        for nbh in range(2):
            ps = ps_s.tile([128, 1024], f32, tag="ps_s", name=f"s{b}_{hp}_{nbh}")
            for nbl in range(2):
                nb = nbh * 2 + nbl
                for h01 in range(2):
                    nc.tensor.matmul(
                        ps[:, h01 * 512 + nbl * 256: h01 * 512 + nbl * 256 + 256],
                        keysT[hp][h01 * 64:(h01 + 1) * 64,
                                  nb * 128:(nb + 1) * 128],
                        qt[h01 * 64:(h01 + 1) * 64, hp * 256:(hp + 1) * 256],
                        start=True, stop=True,
                        tile_position=(h01 * 64, 0),
                        skip_group_check=True,
                    )
            u = upool.tile([128, 1024], bf16, tag="u", name=f"u{b}_{hp}_{nbh}")
            nc.scalar.activation(u[:], ps[:], Exp)
            u_tiles[(b, hp, nbh)] = u

    # ---- ctx + normalize for one 4-head group g covering heads g*4..g*4+3 ----
    out_sbs = {}

    def emit_ctx(b, g):
        values = values_b[b]
        for mb in range(2):
            if (b, mb) not in out_sbs:
                out_sbs[(b, mb)] = opool.tile(
                    [128, 512], bf16, tag="o", name=f"o{b}_{mb}")
            out_sb = out_sbs[(b, mb)]
            ps = ps_c.tile([128, 260], f32, tag="ps_c", name=f"c{b}_{g}_{mb}")
            # each 65-col region is one accumulation chain; chains must not
            # interleave within a tile (start= resets has_written tracking)
            for hh in range(4):
                h = g * 4 + hh
                hp, h01 = h // 2, h % 2
                for nb in range(4):
                    nbh, nbl = nb // 2, nb % 2
                    u = u_tiles[(b, hp, nbh)]
                    nc.tensor.matmul(
                        ps[:, hh * 65: hh * 65 + 65],
                        u[:, h01 * 512 + nbl * 256 + mb * 128:
                          h01 * 512 + nbl * 256 + (mb + 1) * 128],
                        values[nb][:, h * 65:(h + 1) * 65],
                        start=(nb == 0), stop=(nb == 3),
                        skip_group_check=True,
                    )
            view = ps[:].rearrange("p (hh c) -> p hh c", c=65)
            rz = rzpool.tile([128, 4], f32, tag="rz", name=f"rz{b}_{g}_{mb}")
            nc.vector.tensor_scalar_add(rz[:], view[:, :, 64], w_last)
            rzi = rzpool.tile([128, 4], f32, tag="rzi", name=f"rzi{b}_{g}_{mb}")
            nc.vector.reciprocal(rzi[:], rz[:])
            nc.vector.tensor_tensor(
                out_sb[:, g * 256:(g + 1) * 256].rearrange(
                    "p (hh d) -> p hh d", d=64),
                view[:, :, 0:64],
                rzi[:][:, :, None].to_broadcast((128, 4, 64)),
                op=MULT,
            )

    # ---- pipelined emission ----
    # b0: all scores+exp (fills the scalar engine early)
    for hp in range(4):
        emit_scores(0, hp)
    # b1 keys+values projections slot into the PE gap while b0 exps run
    emit_proj(1, nc.vector)
    # b0 ctx for heads 0-3, then b1 scores (keeps scalar fed), then rest
    emit_ctx(0, 0)
    emit_scores(1, 0)
    emit_scores(1, 1)
    emit_ctx(0, 1)
    for mb in range(2):
        nc.gpsimd.dma_start(out_d[0, mb * 128:(mb + 1) * 128, :],
                            out_sbs[(0, mb)][:])
    emit_scores(1, 2)
    emit_scores(1, 3)
    emit_ctx(1, 0)
    emit_ctx(1, 1)
    for mb in range(2):
        nc.gpsimd.dma_start(out_d[1, mb * 128:(mb + 1) * 128, :],
                            out_sbs[(1, mb)][:])


def _get_program(w_last):
    if "nc" not in _CACHE:
        _CACHE["nc"] = _build_program(w_last)
    return _CACHE["nc"]


def make_in_maps(k, q, Wk, Wv):
    import ml_dtypes
    wk, wv, wn, w_last = _host_constants(Wk, Wv)
    k16 = np.asarray(k).astype(ml_dtypes.bfloat16)
    # deinterleave l = 4n+j -> [b, e, j, n] so pooling is unit-stride adds
    k16 = np.ascontiguousarray(
        k16.reshape(B_FULL, E, N, 4).transpose(0, 1, 3, 2)).reshape(B_FULL, E, L)
    qT16 = np.ascontiguousarray(
        np.asarray(q).astype(ml_dtypes.bfloat16).transpose(0, 2, 1))
    in_maps = []
    for c in range(N_CORES):
        in_maps.append({
            "k": np.ascontiguousarray(k16[c * B_PER:(c + 1) * B_PER]),
            "qT": np.ascontiguousarray(qT16[c * B_PER:(c + 1) * B_PER]),
            "wk": wk,
            "wv": wv,
            "wn": wn,
        })
    return in_maps, w_last


def kernel(k, q, Wk, Wv):
    from concourse.bass_utils import run_bass_kernel_spmd

    in_maps, w_last = make_in_maps(k, q, Wk, Wv)
    nc = _get_program(w_last)
    res = run_bass_kernel_spmd(nc, in_maps, core_ids=list(range(N_CORES)))
    out = np.concatenate([res.results[c]["out"] for c in range(N_CORES)], axis=0)
    return out.astype(np.float32)


# revision 7
# speedup vs baseline: 1.1820x; 1.0013x over previous
# Bass/Tile kernel for nn_LongTermAttention (continuous long-term attention
# with rectangular basis functions) on 8 Trainium2 NeuronCores.
#
# Mathematical rewrite (verified exact vs the reference):
#   * G = F^T (F F^T + ridge I)^{-1} for the rectangular basis on the padded
#     uniform grid collapses to G[l, n] = (1/4.5) * [l // 4 == n], so
#     Bc[b,n,e] = (1/4.5) * sum_{j<4} k[b,e,4n+j]  (4-wide sum pooling).
#   * psi on the integration grid is a one-hot selector, so the P=1000-point
#     continuous softmax reduces to basis space with per-basis quadrature
#     mass Wn:  p_n = exp(s_n) Wn_n / Z,  Z = sum_n exp(s_n) Wn_n + w_last,
#     ctx = p @ V.  Wn is folded into the values (V' = Wn V) and into the
#     Z-accumulator column, so the exp needs no bias at all.
#
# Layouts are prepared host-side (free):
#   * k is deinterleaved to [e, j, n] so the 4-wide pooling becomes two
#     unit-stride bf16 adds (DVE 2x mode) instead of stride-2 adds.
#   * q is pre-transposed to [e, t] so no on-chip transpose is needed.
#
# Sharding: data-parallel over batch, 2 batches per core; weights replicated.

import numpy as np

B_FULL = 16
N_CORES = 8
B_PER = B_FULL // N_CORES  # 2
E = 512          # embed dim
L = 2048         # memory length
T = 256          # query length
N = 512          # basis count
H = 8            # heads
D = 64           # head dim
P_GRID = 1000    # integration points
RIDGE_C = 4.5    # F F^T diag (4.0) + ridge (0.5)

_CACHE = {}


def _host_constants(Wk, Wv):
    """Fold pooling normalization (1/4.5) and query scale (1/8) into the
    projection weights; build the per-basis quadrature-mass column."""
    import ml_dtypes
    wk = (Wk.astype(np.float64) / (RIDGE_C * 8.0)).astype(ml_dtypes.bfloat16)
    wv = (Wv.astype(np.float64) / RIDGE_C).astype(ml_dtypes.bfloat16)
    p = np.arange(P_GRID)
    nmap = (512 * p) // 999
    w = np.full(P_GRID, 1.0 / 999.0)
    w[0] = w[-1] = 1.0 / 1998.0
    Wn = np.zeros(N)
    np.add.at(Wn, nmap[:-1], w[:-1])
    wn = np.ascontiguousarray(Wn.astype(np.float32).reshape(4, 128).T)  # [128,4]
    w_last = float(w[-1])
    return wk, wv, wn, w_last


def _build_program(w_last):
    import concourse.bass as bass
    import concourse.mybir as mybir
    import concourse.tile as tile
    from concourse import bacc

    f32 = mybir.dt.float32
    bf16 = mybir.dt.bfloat16

    nc = bacc.Bacc(
        "TRN2",
        target_bir_lowering=False,
        debug=False,
        enable_asserts=False,
        num_devices=N_CORES,
    )

    k_d = nc.dram_tensor("k", [B_PER, E, L], bf16, kind="ExternalInput").ap()
    qT_d = nc.dram_tensor("qT", [B_PER, 128, 4 * T], bf16, kind="ExternalInput").ap()
    wk_d = nc.dram_tensor("wk", [128, 4 * E], bf16, kind="ExternalInput").ap()
    wv_d = nc.dram_tensor("wv", [128, 4 * E], bf16, kind="ExternalInput").ap()
    wn_d = nc.dram_tensor("wn", [128, 4], f32, kind="ExternalInput").ap()
    out_d = nc.dram_tensor("out", [B_PER, T, E], bf16, kind="ExternalOutput").ap()

    from contextlib import ExitStack
    with tile.TileContext(nc) as tc, ExitStack() as ctx:
        _kernel_body(ctx, tc, nc, mybir,
                     k_d, qT_d, wk_d, wv_d, wn_d, out_d, w_last)

    nc.compile()
    return nc


def _kernel_body(ctx, tc, nc, mybir,
                 k_d, qT_d, wk_d, wv_d, wn_d, out_d, w_last):
    f32 = mybir.dt.float32
    bf16 = mybir.dt.bfloat16
    Exp = mybir.ActivationFunctionType.Exp
    MULT = mybir.AluOpType.mult

    def pool(name, bufs, space="SBUF"):
        return ctx.enter_context(tc.tile_pool(name=name, bufs=bufs, space=space))

    consts = pool("consts", 1)
    kpool = pool("kpool", 8)
    t1pool = pool("t1pool", 3)
    plpool = pool("plpool", 8)
    qtpool = pool("qtpool", 2)
    ktpool = pool("ktpool", 8)
    vpool = pool("vpool", 8)
    upool = pool("upool", 12)
    rzpool = pool("rzpool", 4)
    opool = pool("opool", 4)

    ps_proj = pool("ps_proj", 2, "PSUM")   # [128,512] tiles: 1 bank each
    ps_s = pool("ps_s", 2, "PSUM")         # [128,1024] tiles: 2 banks each
    ps_c = pool("ps_c", 2, "PSUM")         # [128,260] tiles: 1 bank each

    # ---- weights first on the two HWDGE rings (sync=qSP, scalar=qAct);
    #      host pre-packs them so every DMA has 4KB-contiguous rows ----
    wk_sb = consts.tile([128, 4 * 512], bf16, tag="wk")  # [e%128, kk*512+e']
    wv_sb = consts.tile([128, 4 * 512], bf16, tag="wv")
    wn_sb = consts.tile([128, 4], f32, tag="wn")
    nc.sync.dma_start(wk_sb[:], wk_d[:])
    nc.scalar.dma_start(wv_sb[:], wv_d[:])
    nc.scalar.dma_start(wn_sb[:], wn_d[:])

    # ---- k DMA (both rings) + qT DMA; k cols are (j, n) deinterleaved ----
    kts = {}
    qt_b = []
    for b in range(B_PER):
        for et in range(4):
            ring = nc.sync if et % 2 == 0 else nc.scalar
            kt = kpool.tile([128, L], bf16, tag="k", name=f"kt{b}_{et}")
            ring.dma_start(kt[:], k_d[b, et * 128:(et + 1) * 128, :])
            kts[(b, et)] = kt
        qt = qtpool.tile([128, 4 * T], bf16, tag="qt", name=f"qt{b}")
        nc.sync.dma_start(qt[:], qT_d[b])
        qt_b.append(qt)

    # ---- pooling: two unit-stride bf16 adds per k tile (vector; et3->gpsimd) ----
    pooled_b = [[None] * 4 for _ in range(B_PER)]

    def emit_pool(b, et):
        kt = kts[(b, et)]
        eng = nc.gpsimd if et == 3 else nc.vector
        t1 = t1pool.tile([128, L // 2], bf16, tag="t1", name=f"t1_{b}_{et}")
        eng.tensor_add(t1[:], kt[:, 0:1024], kt[:, 1024:2048])
        pl = plpool.tile([128, N], bf16, tag="pl", name=f"pl{b}_{et}")
        eng.tensor_add(pl[:], t1[:, 0:512], t1[:, 512:1024])
        pooled_b[b][et] = pl

    for et in range(4):
        emit_pool(0, et)

    # ---- projections for a batch ----
    keysT_b = [[None] * 4 for _ in range(B_PER)]
    values_b = [[None] * 4 for _ in range(B_PER)]

    def emit_proj(b, kt_copy_eng):
        pooled = pooled_b[b]
        for m in range(4):
            # keysT[m] = wk^T @ pooled -> [e' (block m), n]
            ps = ps_proj.tile([128, 512], f32, tag="pp", name=f"psk{b}_{m}")
            for kk in range(4):
                nc.tensor.matmul(
                    ps[:],
                    wk_sb[:, kk * 512 + m * 128: kk * 512 + (m + 1) * 128],
                    pooled[kk][:],
                    start=(kk == 0), stop=(kk == 3),
                )
            kT = ktpool.tile([128, 512], bf16, tag="kT", name=f"kT{b}_{m}")
            if kt_copy_eng is nc.scalar:
                nc.scalar.copy(kT[:], ps[:])
            else:
                kt_copy_eng.tensor_copy(kT[:], ps[:])
            keysT_b[b][m] = kT

            # values[m] = pooled^T @ wv -> [n (block m), e'], scaled by Wn,
            # with the quadrature mass as a 65th column per head.
            ps2 = ps_proj.tile([128, 512], f32, tag="pp", name=f"psv{b}_{m}")
            for kk in range(4):
                nc.tensor.matmul(
                    ps2[:],
                    pooled[kk][:, m * 128:(m + 1) * 128],
                    wv_sb[:, kk * 512:(kk + 1) * 512],
                    start=(kk == 0), stop=(kk == 3),
                )
            v_sb = vpool.tile([128, 8 * 65], bf16, tag="v", name=f"v{b}_{m}")
            vv = v_sb[:].rearrange("p (h c) -> p h c", c=65)
            nc.vector.tensor_scalar_mul(
                vv[:, :, 0:64],
                ps2[:].rearrange("p (h d) -> p h d", d=64),
                wn_sb[:, m:m + 1])
            nc.vector.tensor_copy(vv[:, :, 64], wn_sb[:, m:m + 1].to_broadcast((128, 8)))
            values_b[b][m] = v_sb

    emit_proj(0, nc.scalar)

    for et in range(4):
        emit_pool(1, et)

    # ---- scores + exp for one head-pair hp: u[n, (h01, nbl, t)] tiles ----
    u_tiles = {}

    def emit_scores(b, hp):
        keysT = keysT_b[b]
        qt = qt_b[b]
        for nbh in range(2):
            ps = ps_s.tile([128, 1024], f32, tag="ps_s", name=f"s{b}_{hp}_{nbh}")
            for nbl in range(2):
                nb = nbh * 2 + nbl
                for h01 in range(2):
                    nc.tensor.matmul(
                        ps[:, h01 * 512 + nbl * 256: h01 * 512 + nbl * 256 + 256],
                        keysT[hp][h01 * 64:(h01 + 1) * 64,
                                  nb * 128:(nb + 1) * 128],
                        qt[h01 * 64:(h01 + 1) * 64, hp * 256:(hp + 1) * 256],
                        start=True, stop=True,
                        tile_position=(h01 * 64, 0),
                        skip_group_check=True,
                    )
            u = upool.tile([128, 1024], bf16, tag="u", name=f"u{b}_{hp}_{nbh}")
            nc.scalar.activation(u[:], ps[:], Exp)
            u_tiles[(b, hp, nbh)] = u

    # ---- ctx + normalize for one 4-head group g covering heads g*4..g*4+3 ----
    out_sbs = {}

    def emit_ctx(b, g):
        values = values_b[b]
        for mb in range(2):
            if (b, mb) not in out_sbs:
                out_sbs[(b, mb)] = opool.tile(
                    [128, 512], bf16, tag="o", name=f"o{b}_{mb}")
            out_sb = out_sbs[(b, mb)]
            ps = ps_c.tile([128, 260], f32, tag="ps_c", name=f"c{b}_{g}_{mb}")
            # each 65-col region is one accumulation chain; chains must not
            # interleave within a tile (start= resets has_written tracking)
            for hh in range(4):
                h = g * 4 + hh
                hp, h01 = h // 2, h % 2
                for nb in range(4):
                    nbh, nbl = nb // 2, nb % 2
                    u = u_tiles[(b, hp, nbh)]
                    nc.tensor.matmul(
                        ps[:, hh * 65: hh * 65 + 65],
                        u[:, h01 * 512 + nbl * 256 + mb * 128:
                          h01 * 512 + nbl * 256 + (mb + 1) * 128],
                        values[nb][:, h * 65:(h + 1) * 65],
                        start=(nb == 0), stop=(nb == 3),
                        skip_group_check=True,
                    )
            view = ps[:].rearrange("p (hh c) -> p hh c", c=65)
            rz = rzpool.tile([128, 4], f32, tag="rz", name=f"rz{b}_{g}_{mb}")
            nc.vector.tensor_scalar_add(rz[:], view[:, :, 64], w_last)
            rzi = rzpool.tile([128, 4], f32, tag="rzi", name=f"rzi{b}_{g}_{mb}")
            nc.vector.reciprocal(rzi[:], rz[:])
            nc.vector.tensor_tensor(
                out_sb[:, g * 256:(g + 1) * 256].rearrange(
                    "p (hh d) -> p hh d", d=64),
                view[:, :, 0:64],
                rzi[:][:, :, None].to_broadcast((128, 4, 64)),
                op=MULT,
            )

    # ---- pipelined emission ----
    # b0: all scores+exp (fills the scalar engine early)
    for hp in range(4):
        emit_scores(0, hp)
    # b1 keys+values projections slot into the PE gap while b0 exps run
    emit_proj(1, nc.vector)
    # b0 ctx for heads 0-3, then b1 scores (keeps scalar fed), then rest
    emit_ctx(0, 0)
    emit_scores(1, 0)
    emit_scores(1, 1)
    emit_ctx(0, 1)
    for mb in range(2):
        nc.gpsimd.dma_start(out_d[0, mb * 128:(mb + 1) * 128, :],
                            out_sbs[(0, mb)][:])
    emit_scores(1, 2)
    emit_scores(1, 3)
    emit_ctx(1, 0)
    emit_ctx(1, 1)
    for mb in range(2):
        nc.gpsimd.dma_start(out_d[1, mb * 128:(mb + 1) * 128, :],
                            out_sbs[(1, mb)][:])


def _get_program(w_last):
    if "nc" not in _CACHE:
        _CACHE["nc"] = _build_program(w_last)
    return _CACHE["nc"]


def make_in_maps(k, q, Wk, Wv):
    import ml_dtypes
    wk, wv, wn, w_last = _host_constants(Wk, Wv)
    k16 = np.asarray(k).astype(ml_dtypes.bfloat16)
    # deinterleave l = 4n+j -> [b, e, j, n] so pooling is unit-stride adds
    k16 = np.ascontiguousarray(
        k16.reshape(B_FULL, E, N, 4).transpose(0, 1, 3, 2)).reshape(B_FULL, E, L)
    # qT packed to match SBUF layout [p, eb*256+t]: row p holds q^T rows
    # eb*128+p for eb=0..3 -> 2KB-contiguous DMA rows
    qT16 = np.asarray(q).astype(ml_dtypes.bfloat16).transpose(0, 2, 1)  # [B,E,T]
    qT16 = np.ascontiguousarray(
        qT16.reshape(B_FULL, 4, 128, T).transpose(0, 2, 1, 3).reshape(
            B_FULL, 128, 4 * T))
    # wk/wv packed to SBUF layout [p, kk*512+e'] (row e = kk*128+p)
    wk = np.ascontiguousarray(
        wk.reshape(4, 128, E).transpose(1, 0, 2).reshape(128, 4 * E))
    wv = np.ascontiguousarray(
        wv.reshape(4, 128, E).transpose(1, 0, 2).reshape(128, 4 * E))
    in_maps = []
    for c in range(N_CORES):
        in_maps.append({
            "k": np.ascontiguousarray(k16[c * B_PER:(c + 1) * B_PER]),
            "qT": np.ascontiguousarray(qT16[c * B_PER:(c + 1) * B_PER]),
            "wk": wk,
            "wv": wv,
            "wn": wn,
        })
    return in_maps, w_last


def kernel(k, q, Wk, Wv):
    from concourse.bass_utils import run_bass_kernel_spmd

    in_maps, w_last = make_in_maps(k, q, Wk, Wv)
    nc = _get_program(w_last)
    res = run_bass_kernel_spmd(nc, in_maps, core_ids=list(range(N_CORES)))
    out = np.concatenate([res.results[c]["out"] for c in range(N_CORES)], axis=0)
    return out.astype(np.float32)


# revision 11
# speedup vs baseline: 1.1932x; 1.0094x over previous
# Bass/Tile kernel for nn_LongTermAttention (continuous long-term attention
# with rectangular basis functions) on 8 Trainium2 NeuronCores.
#
# Mathematical rewrite (verified exact vs the reference):
#   * G = F^T (F F^T + ridge I)^{-1} for the rectangular basis on the padded
#     uniform grid collapses to G[l, n] = (1/4.5) * [l // 4 == n], so
#     Bc[b,n,e] = (1/4.5) * sum_{j<4} k[b,e,4n+j]  (4-wide sum pooling).
#   * psi on the integration grid is a one-hot selector, so the P=1000-point
#     continuous softmax reduces to basis space with per-basis quadrature
#     mass Wn:  p_n = exp(s_n) Wn_n / Z,  Z = sum_n exp(s_n) Wn_n + w_last,
#     ctx = p @ V.  Wn is folded into the values (V' = Wn V) and into the
#     Z-accumulator column, so the exp needs no bias at all.
#
# Layouts are prepared host-side (free):
#   * k is deinterleaved to [e, j, n] so the 4-wide pooling becomes two
#     unit-stride bf16 adds (DVE 2x mode) instead of stride-2 adds.
#   * q is pre-transposed to [e, t] so no on-chip transpose is needed.
#
# Sharding: data-parallel over batch, 2 batches per core; weights replicated.

import numpy as np

B_FULL = 16
N_CORES = 8
B_PER = B_FULL // N_CORES  # 2
E = 512          # embed dim
L = 2048         # memory length
T = 256          # query length
N = 512          # basis count
H = 8            # heads
D = 64           # head dim
P_GRID = 1000    # integration points
RIDGE_C = 4.5    # F F^T diag (4.0) + ridge (0.5)

_CACHE = {}


def _host_constants(Wk, Wv):
    """Fold pooling normalization (1/4.5) and query scale (1/8) into the
    projection weights; build the per-basis quadrature-mass column."""
    import ml_dtypes
    wk = (Wk.astype(np.float64) / (RIDGE_C * 8.0)).astype(ml_dtypes.bfloat16)
    wv = (Wv.astype(np.float64) / RIDGE_C).astype(ml_dtypes.bfloat16)
    p = np.arange(P_GRID)
    nmap = (512 * p) // 999
    w = np.full(P_GRID, 1.0 / 999.0)
    w[0] = w[-1] = 1.0 / 1998.0
    Wn = np.zeros(N)
    np.add.at(Wn, nmap[:-1], w[:-1])
    wn = np.ascontiguousarray(Wn.astype(np.float32).reshape(4, 128).T)  # [128,4]
    w_last = float(w[-1])
    return wk, wv, wn, w_last


def _build_program(w_last):
    import concourse.bass as bass
    import concourse.mybir as mybir
    import concourse.tile as tile
    from concourse import bacc

    f32 = mybir.dt.float32
    bf16 = mybir.dt.bfloat16

    nc = bacc.Bacc(
        "TRN2",
        target_bir_lowering=False,
        debug=False,
        enable_asserts=False,
        num_devices=N_CORES,
    )

    k_d = nc.dram_tensor("k", [B_PER, E, L], bf16, kind="ExternalInput").ap()
    qT_d = nc.dram_tensor("qT", [B_PER, 128, 4 * T], bf16, kind="ExternalInput").ap()
    wk_d = nc.dram_tensor("wk", [128, 4 * E], bf16, kind="ExternalInput").ap()
    wv_d = nc.dram_tensor("wv", [128, 4 * E], bf16, kind="ExternalInput").ap()
    wn_d = nc.dram_tensor("wn", [128, 4], f32, kind="ExternalInput").ap()
    out_d = nc.dram_tensor("out", [B_PER, T, E], bf16, kind="ExternalOutput").ap()

    from contextlib import ExitStack
    with tile.TileContext(nc) as tc, ExitStack() as ctx:
        _kernel_body(ctx, tc, nc, mybir,
                     k_d, qT_d, wk_d, wv_d, wn_d, out_d, w_last)

    nc.compile()
    return nc


def _kernel_body(ctx, tc, nc, mybir,
                 k_d, qT_d, wk_d, wv_d, wn_d, out_d, w_last):
    f32 = mybir.dt.float32
    bf16 = mybir.dt.bfloat16
    Exp = mybir.ActivationFunctionType.Exp
    MULT = mybir.AluOpType.mult

    def pool(name, bufs, space="SBUF"):
        return ctx.enter_context(tc.tile_pool(name=name, bufs=bufs, space=space))

    consts = pool("consts", 1)
    kpool = pool("kpool", 8)
    t1pool = pool("t1pool", 3)
    plpool = pool("plpool", 8)
    qtpool = pool("qtpool", 2)
    ktpool = pool("ktpool", 8)
    vpool = pool("vpool", 8)
    upool = pool("upool", 12)
    rzpool = pool("rzpool", 4)
    opool = pool("opool", 4)

    ps_proj = pool("ps_proj", 2, "PSUM")   # [128,512] tiles: 1 bank each
    ps_s = pool("ps_s", 2, "PSUM")         # [128,1024] tiles: 2 banks each
    ps_c = pool("ps_c", 2, "PSUM")         # [128,260] tiles: 1 bank each

    # ---- DMA plan: only sync(qSP) and scalar(qAct) have HWDGE rings.
    #      Priority order: qT-b0 + wk (gate scores) and k-b0 first; wv/wn
    #      before values proj; k-b1 + qT-b1 after; outputs at the end. ----
    wk_sb = consts.tile([128, 4 * 512], bf16, tag="wk")  # [e%128, kk*512+e']
    wv_sb = consts.tile([128, 4 * 512], bf16, tag="wv")
    wn_sb = consts.tile([128, 4], f32, tag="wn")
    kts = {}
    qt_b = []

    def dma_k(b, et, ring):
        kt = kpool.tile([128, L], bf16, tag="k", name=f"kt{b}_{et}")
        ring.dma_start(kt[:], k_d[b, et * 128:(et + 1) * 128, :])
        kts[(b, et)] = kt

    def dma_qt(b, ring):
        qt = qtpool.tile([128, 4 * T], bf16, tag="qt", name=f"qt{b}")
        ring.dma_start(qt[:], qT_d[b])
        qt_b.append(qt)

    dma_qt(0, nc.sync)
    nc.sync.dma_start(wk_sb[:], wk_d[:])
    dma_k(0, 2, nc.scalar)
    dma_k(0, 3, nc.scalar)
    dma_k(0, 0, nc.sync)
    dma_k(0, 1, nc.sync)
    nc.scalar.dma_start(wn_sb[:], wn_d[:])
    nc.scalar.dma_start(wv_sb[:], wv_d[:])
    dma_qt(1, nc.sync)
    dma_k(1, 2, nc.scalar)
    dma_k(1, 0, nc.sync)
    dma_k(1, 3, nc.scalar)
    dma_k(1, 1, nc.sync)

    # ---- pooling: two unit-stride bf16 adds per k tile, all on vector,
    #      emitted in expected DMA-arrival order ----
    pooled_b = [[None] * 4 for _ in range(B_PER)]

    def emit_pool(b, et):
        kt = kts[(b, et)]
        t1 = t1pool.tile([128, L // 2], bf16, tag="t1", name=f"t1_{b}_{et}")
        nc.vector.tensor_add(t1[:], kt[:, 0:1024], kt[:, 1024:2048])
        pl = plpool.tile([128, N], bf16, tag="pl", name=f"pl{b}_{et}")
        nc.vector.tensor_add(pl[:], t1[:, 0:512], t1[:, 512:1024])
        pooled_b[b][et] = pl

    for et in (2, 0, 3, 1):
        emit_pool(0, et)

    # ---- projections for a batch ----
    keysT_b = [[None] * 4 for _ in range(B_PER)]
    values_b = [[None] * 4 for _ in range(B_PER)]

    def emit_keys(b, m, copy_eng):
        # keysT[m] = wk^T @ pooled -> [e' (block m), n]
        pooled = pooled_b[b]
        ps = ps_proj.tile([128, 512], f32, tag="pp", name=f"psk{b}_{m}")
        for kk in range(4):
            nc.tensor.matmul(
                ps[:],
                wk_sb[:, kk * 512 + m * 128: kk * 512 + (m + 1) * 128],
                pooled[kk][:],
                start=(kk == 0), stop=(kk == 3),
            )
        kT = ktpool.tile([128, 512], bf16, tag="kT", name=f"kT{b}_{m}")
        if copy_eng is nc.scalar:
            nc.scalar.copy(kT[:], ps[:])
        else:
            copy_eng.tensor_copy(kT[:], ps[:])
        keysT_b[b][m] = kT

    def emit_values(b, m):
        # values[m] = pooled^T @ wv -> [n (block m), e'], scaled by Wn,
        # with the quadrature mass as a 65th column per head.
        pooled = pooled_b[b]
        ps2 = ps_proj.tile([128, 512], f32, tag="pp", name=f"psv{b}_{m}")
        for kk in range(4):
            nc.tensor.matmul(
                ps2[:],
                pooled[kk][:, m * 128:(m + 1) * 128],
                wv_sb[:, kk * 512:(kk + 1) * 512],
                start=(kk == 0), stop=(kk == 3),
            )
        v_sb = vpool.tile([128, 8 * 65], bf16, tag="v", name=f"v{b}_{m}")
        vv = v_sb[:].rearrange("p (h c) -> p h c", c=65)
        nc.vector.tensor_scalar_mul(
            vv[:, :, 0:64],
            ps2[:].rearrange("p (h d) -> p h d", d=64),
            wn_sb[:, m:m + 1])
        nc.vector.tensor_copy(vv[:, :, 64], wn_sb[:, m:m + 1].to_broadcast((128, 8)))
        values_b[b][m] = v_sb

    # ---- scores + exp for one head-pair hp: u[n, (h01, nbl, t)] tiles ----
    u_tiles = {}

    def emit_scores(b, hp):
        keysT = keysT_b[b]
        qt = qt_b[b]
        for nbh in range(2):
            ps = ps_s.tile([128, 1024], f32, tag="ps_s", name=f"s{b}_{hp}_{nbh}")
            for nbl in range(2):
                nb = nbh * 2 + nbl
                for h01 in range(2):
                    nc.tensor.matmul(
                        ps[:, h01 * 512 + nbl * 256: h01 * 512 + nbl * 256 + 256],
                        keysT[hp][h01 * 64:(h01 + 1) * 64,
                                  nb * 128:(nb + 1) * 128],
                        qt[h01 * 64:(h01 + 1) * 64, hp * 256:(hp + 1) * 256],
                        start=True, stop=True,
                        tile_position=(h01 * 64, 0),
                        skip_group_check=True,
                    )
            u = upool.tile([128, 1024], bf16, tag="u", name=f"u{b}_{hp}_{nbh}")
            nc.scalar.activation(u[:], ps[:], Exp)
            u_tiles[(b, hp, nbh)] = u

    # ---- ctx + normalize for one 4-head group g covering heads g*4..g*4+3 ----
    out_sbs = {}

    def emit_ctx(b, g):
        values = values_b[b]
        for mb in range(2):
            if (b, mb) not in out_sbs:
                out_sbs[(b, mb)] = opool.tile(
                    [128, 512], bf16, tag="o", name=f"o{b}_{mb}")
            out_sb = out_sbs[(b, mb)]
            ps = ps_c.tile([128, 260], f32, tag="ps_c", name=f"c{b}_{g}_{mb}")
            # each 65-col region is one accumulation chain; chains must not
            # interleave within a tile (start= resets has_written tracking)
            for hh in range(4):
                h = g * 4 + hh
                hp, h01 = h // 2, h % 2
                for nb in range(4):
                    nbh, nbl = nb // 2, nb % 2
                    u = u_tiles[(b, hp, nbh)]
                    nc.tensor.matmul(
                        ps[:, hh * 65: hh * 65 + 65],
                        u[:, h01 * 512 + nbl * 256 + mb * 128:
                          h01 * 512 + nbl * 256 + (mb + 1) * 128],
                        values[nb][:, h * 65:(h + 1) * 65],
                        start=(nb == 0), stop=(nb == 3),
                        skip_group_check=True,
                    )
            view = ps[:].rearrange("p (hh c) -> p hh c", c=65)
            # w_last/Z <= 5e-4, so the +w_last term is dropped (validated)
            rzi = rzpool.tile([128, 4], f32, tag="rzi", name=f"rzi{b}_{g}_{mb}")
            nc.vector.reciprocal(rzi[:], view[:, :, 64])
            nc.vector.tensor_tensor(
                out_sb[:, g * 256:(g + 1) * 256].rearrange(
                    "p (hh d) -> p hh d", d=64),
                view[:, :, 0:64],
                rzi[:][:, :, None].to_broadcast((128, 4, 64)),
                op=MULT,
            )

    # ---- pipelined emission ----
    # b0: interleave keys-proj with scores per m-block so the first exp
    # fires as soon as keysT[0] exists; m0 copy on the (idle) scalar,
    # m1-3 on vector.
    emit_keys(0, 0, nc.scalar)
    emit_scores(0, 0)
    for m in range(1, 4):
        emit_keys(0, m, nc.vector)
        emit_scores(0, m)
    for m in range(4):
        emit_values(0, m)
    for et in (2, 0, 3, 1):
        emit_pool(1, et)
    emit_ctx(0, 0)
    for m in range(4):
        emit_keys(1, m, nc.vector)
    for m in range(4):
        emit_values(1, m)
    emit_scores(1, 0)
    emit_scores(1, 1)
    emit_ctx(0, 1)
    for mb in range(2):
        nc.sync.dma_start(out_d[0, mb * 128:(mb + 1) * 128, :],
                          out_sbs[(0, mb)][:])
    emit_scores(1, 2)
    emit_scores(1, 3)
    emit_ctx(1, 0)
    emit_ctx(1, 1)
    for mb in range(2):
        nc.scalar.dma_start(out_d[1, mb * 128:(mb + 1) * 128, :],
                            out_sbs[(1, mb)][:])


def _get_program(w_last):
    if "nc" not in _CACHE:
        _CACHE["nc"] = _build_program(w_last)
    return _CACHE["nc"]


def make_in_maps(k, q, Wk, Wv):
    import ml_dtypes
    wk, wv, wn, w_last = _host_constants(Wk, Wv)
    k16 = np.asarray(k).astype(ml_dtypes.bfloat16)
    # deinterleave l = 4n+j -> [b, e, j, n] so pooling is unit-stride adds
    k16 = np.ascontiguousarray(
        k16.reshape(B_FULL, E, N, 4).transpose(0, 1, 3, 2)).reshape(B_FULL, E, L)
    # qT packed to match SBUF layout [p, eb*256+t]: row p holds q^T rows
    # eb*128+p for eb=0..3 -> 2KB-contiguous DMA rows
    qT16 = np.asarray(q).astype(ml_dtypes.bfloat16).transpose(0, 2, 1)  # [B,E,T]
    qT16 = np.ascontiguousarray(
        qT16.reshape(B_FULL, 4, 128, T).transpose(0, 2, 1, 3).reshape(
            B_FULL, 128, 4 * T))
    # wk/wv packed to SBUF layout [p, kk*512+e'] (row e = kk*128+p)
    wk = np.ascontiguousarray(
        wk.reshape(4, 128, E).transpose(1, 0, 2).reshape(128, 4 * E))
    wv = np.ascontiguousarray(
        wv.reshape(4, 128, E).transpose(1, 0, 2).reshape(128, 4 * E))
    in_maps = []
    for c in range(N_CORES):
        in_maps.append({
            "k": np.ascontiguousarray(k16[c * B_PER:(c + 1) * B_PER]),
            "qT": np.ascontiguousarray(qT16[c * B_PER:(c + 1) * B_PER]),
            "wk": wk,
            "wv": wv,
            "wn": wn,
        })
    return in_maps, w_last


def kernel(k, q, Wk, Wv):
    from concourse.bass_utils import run_bass_kernel_spmd

    in_maps, w_last = make_in_maps(k, q, Wk, Wv)
    nc = _get_program(w_last)
    res = run_bass_kernel_spmd(nc, in_maps, core_ids=list(range(N_CORES)))
    out = np.concatenate([res.results[c]["out"] for c in range(N_CORES)], axis=0)
    return out.astype(np.float32)


# revision 14
# speedup vs baseline: 1.2775x; 1.0707x over previous
# Bass/Tile kernel for nn_LongTermAttention (continuous long-term attention
# with rectangular basis functions) on 8 Trainium2 NeuronCores.
#
# Mathematical rewrite (verified exact vs the reference):
#   * G = F^T (F F^T + ridge I)^{-1} for the rectangular basis on the padded
#     uniform grid collapses to G[l, n] = (1/4.5) * [l // 4 == n], so
#     Bc[b,n,e] = (1/4.5) * sum_{j<4} k[b,e,4n+j]  (4-wide sum pooling).
#   * psi on the integration grid is a one-hot selector, so the P=1000-point
#     continuous softmax reduces to basis space with per-basis quadrature
#     mass Wn:  p_n = exp(s_n) Wn_n / Z,  Z = sum_n exp(s_n) Wn_n + w_last,
#     ctx = p @ V.  Wn is folded into the values (V' = Wn V) and into the
#     Z-accumulator column, so the exp needs no bias at all.
#
# Layouts are prepared host-side (free):
#   * k is deinterleaved to [e, j, n] so the 4-wide pooling becomes two
#     unit-stride bf16 adds (DVE 2x mode) instead of stride-2 adds.
#   * q is pre-transposed to [e, t] so no on-chip transpose is needed.
#
# Sharding: data-parallel over batch, 2 batches per core; weights replicated.

import numpy as np

B_FULL = 16
N_CORES = 8
B_PER = B_FULL // N_CORES  # 2
E = 512          # embed dim
L = 2048         # memory length
T = 256          # query length
N = 512          # basis count
H = 8            # heads
D = 64           # head dim
P_GRID = 1000    # integration points
RIDGE_C = 4.5    # F F^T diag (4.0) + ridge (0.5)

_CACHE = {}


def _host_constants(Wk, Wv):
    """Fold pooling normalization (1/4.5) and query scale (1/8) into the
    projection weights; build the per-basis quadrature-mass column."""
    import ml_dtypes
    wk = (Wk.astype(np.float64) / (RIDGE_C * 8.0)).astype(ml_dtypes.bfloat16)
    wv = (Wv.astype(np.float64) / RIDGE_C).astype(ml_dtypes.bfloat16)
    p = np.arange(P_GRID)
    nmap = (512 * p) // 999
    w = np.full(P_GRID, 1.0 / 999.0)
    w[0] = w[-1] = 1.0 / 1998.0
    Wn = np.zeros(N)
    np.add.at(Wn, nmap[:-1], w[:-1])
    wn = np.ascontiguousarray(Wn.astype(np.float32).reshape(4, 128).T)  # [128,4]
    w_last = float(w[-1])
    return wk, wv, wn, w_last


def _build_program(w_last):
    import concourse.bass as bass
    import concourse.mybir as mybir
    import concourse.tile as tile
    from concourse import bacc

    f32 = mybir.dt.float32
    bf16 = mybir.dt.bfloat16

    nc = bacc.Bacc(
        "TRN2",
        target_bir_lowering=False,
        debug=False,
        enable_asserts=False,
        num_devices=N_CORES,
    )

    k_d = nc.dram_tensor("k", [B_PER, E, L], bf16, kind="ExternalInput").ap()
    qT_d = nc.dram_tensor("qT", [B_PER, 128, 4 * T], bf16, kind="ExternalInput").ap()
    wk_d = nc.dram_tensor("wk", [128, 4 * E], bf16, kind="ExternalInput").ap()
    wv_d = nc.dram_tensor("wv", [128, 4 * E], bf16, kind="ExternalInput").ap()
    wn_d = nc.dram_tensor("wn", [128, 4], f32, kind="ExternalInput").ap()
    out_d = nc.dram_tensor("out", [B_PER, T, E], bf16, kind="ExternalOutput").ap()

    from contextlib import ExitStack
    with tile.TileContext(nc) as tc, ExitStack() as ctx:
        _kernel_body(ctx, tc, nc, mybir,
                     k_d, qT_d, wk_d, wv_d, wn_d, out_d, w_last)

    nc.compile()
    return nc


def _kernel_body(ctx, tc, nc, mybir,
                 k_d, qT_d, wk_d, wv_d, wn_d, out_d, w_last):
    f32 = mybir.dt.float32
    bf16 = mybir.dt.bfloat16
    Exp = mybir.ActivationFunctionType.Exp
    MULT = mybir.AluOpType.mult

    def pool(name, bufs, space="SBUF"):
        return ctx.enter_context(tc.tile_pool(name=name, bufs=bufs, space=space))

    consts = pool("consts", 1)
    kpool = pool("kpool", 8)
    t1pool = pool("t1pool", 3)
    plpool = pool("plpool", 8)
    qtpool = pool("qtpool", 2)
    ktpool = pool("ktpool", 8)
    vpool = pool("vpool", 8)
    upool = pool("upool", 12)
    rzpool = pool("rzpool", 4)
    opool = pool("opool", 4)

    ps_proj = pool("ps_proj", 2, "PSUM")   # [128,512] tiles: 1 bank each
    ps_s = pool("ps_s", 2, "PSUM")         # [128,1024] tiles: 2 banks each
    ps_c = pool("ps_c", 2, "PSUM")         # [128,260] tiles: 1 bank each

    # ---- DMA plan: only sync(qSP) and scalar(qAct) have HWDGE rings.
    #      Priority order: qT-b0 + wk (gate scores) and k-b0 first; wv/wn
    #      before values proj; k-b1 + qT-b1 after; outputs at the end. ----
    wk_sb = consts.tile([128, 4 * 512], bf16, tag="wk")  # [e%128, kk*512+e']
    wv_sb = consts.tile([128, 4 * 512], bf16, tag="wv")
    wn_sb = consts.tile([128, 4], f32, tag="wn")
    kts = {}
    qt_b = []

    def dma_k(b, et, ring):
        kt = kpool.tile([128, L], bf16, tag="k", name=f"kt{b}_{et}")
        ring.dma_start(kt[:], k_d[b, et * 128:(et + 1) * 128, :])
        kts[(b, et)] = kt

    def dma_qt(b, ring):
        qt = qtpool.tile([128, 4 * T], bf16, tag="qt", name=f"qt{b}")
        ring.dma_start(qt[:], qT_d[b])
        qt_b.append(qt)

    dma_qt(0, nc.sync)
    nc.scalar.dma_start(wk_sb[:], wk_d[:])
    dma_k(0, 0, nc.sync)
    dma_k(0, 1, nc.sync)
    dma_k(0, 2, nc.scalar)
    dma_k(0, 3, nc.sync)
    nc.scalar.dma_start(wn_sb[:], wn_d[:])
    nc.scalar.dma_start(wv_sb[:], wv_d[:])
    dma_qt(1, nc.sync)
    dma_k(1, 0, nc.sync)
    dma_k(1, 2, nc.scalar)
    dma_k(1, 1, nc.sync)
    dma_k(1, 3, nc.scalar)

    # ---- pooling: two unit-stride bf16 adds per k tile, all on vector,
    #      emitted in expected DMA-arrival order ----
    pooled_b = [[None] * 4 for _ in range(B_PER)]

    def emit_pool(b, et):
        kt = kts[(b, et)]
        t1 = t1pool.tile([128, L // 2], bf16, tag="t1", name=f"t1_{b}_{et}")
        nc.vector.tensor_add(t1[:], kt[:, 0:1024], kt[:, 1024:2048])
        pl = plpool.tile([128, N], bf16, tag="pl", name=f"pl{b}_{et}")
        nc.vector.tensor_add(pl[:], t1[:, 0:512], t1[:, 512:1024])
        pooled_b[b][et] = pl

    for et in (0, 1, 2, 3):
        emit_pool(0, et)

    # ---- projections for a batch ----
    keysT_b = [[None] * 4 for _ in range(B_PER)]
    values_b = [[None] * 4 for _ in range(B_PER)]

    def emit_keys(b, m, copy_eng):
        # keysT[m] = wk^T @ pooled -> [e' (block m), n]
        pooled = pooled_b[b]
        ps = ps_proj.tile([128, 512], f32, tag="pp", name=f"psk{b}_{m}")
        for kk in range(4):
            nc.tensor.matmul(
                ps[:],
                wk_sb[:, kk * 512 + m * 128: kk * 512 + (m + 1) * 128],
                pooled[kk][:],
                start=(kk == 0), stop=(kk == 3),
            )
        kT = ktpool.tile([128, 512], bf16, tag="kT", name=f"kT{b}_{m}")
        if copy_eng is nc.scalar:
            nc.scalar.copy(kT[:], ps[:])
        else:
            copy_eng.tensor_copy(kT[:], ps[:])
        keysT_b[b][m] = kT

    def emit_values(b, m):
        # values[m] = pooled^T @ wv -> [n (block m), e'], scaled by Wn,
        # with the quadrature mass as a 65th column per head.
        pooled = pooled_b[b]
        ps2 = ps_proj.tile([128, 512], f32, tag="pp", name=f"psv{b}_{m}")
        for kk in range(4):
            nc.tensor.matmul(
                ps2[:],
                pooled[kk][:, m * 128:(m + 1) * 128],
                wv_sb[:, kk * 512:(kk + 1) * 512],
                start=(kk == 0), stop=(kk == 3),
            )
        v_sb = vpool.tile([128, 8 * 65], bf16, tag="v", name=f"v{b}_{m}")
        vv = v_sb[:].rearrange("p (h c) -> p h c", c=65)
        nc.vector.tensor_scalar_mul(
            vv[:, :, 0:64],
            ps2[:].rearrange("p (h d) -> p h d", d=64),
            wn_sb[:, m:m + 1])
        nc.vector.tensor_copy(vv[:, :, 64], wn_sb[:, m:m + 1].to_broadcast((128, 8)))
        values_b[b][m] = v_sb

    # ---- scores + exp for one head-pair hp: u[n, (h01, nbl, t)] tiles ----
    u_tiles = {}

    def emit_scores(b, hp):
        keysT = keysT_b[b]
        qt = qt_b[b]
        for nbh in range(2):
            ps = ps_s.tile([128, 1024], f32, tag="ps_s", name=f"s{b}_{hp}_{nbh}")
            for nbl in range(2):
                nb = nbh * 2 + nbl
                for h01 in range(2):
                    nc.tensor.matmul(
                        ps[:, h01 * 512 + nbl * 256: h01 * 512 + nbl * 256 + 256],
                        keysT[hp][h01 * 64:(h01 + 1) * 64,
                                  nb * 128:(nb + 1) * 128],
                        qt[h01 * 64:(h01 + 1) * 64, hp * 256:(hp + 1) * 256],
                        start=True, stop=True,
                        tile_position=(h01 * 64, 0),
                        skip_group_check=True,
                    )
            u = upool.tile([128, 1024], bf16, tag="u", name=f"u{b}_{hp}_{nbh}")
            nc.scalar.activation(u[:], ps[:], Exp)
            u_tiles[(b, hp, nbh)] = u

    # ---- ctx + normalize for one 4-head group g covering heads g*4..g*4+3 ----
    out_sbs = {}

    def emit_ctx(b, g):
        values = values_b[b]
        for mb in range(2):
            if (b, mb) not in out_sbs:
                out_sbs[(b, mb)] = opool.tile(
                    [128, 512], bf16, tag="o", name=f"o{b}_{mb}")
            out_sb = out_sbs[(b, mb)]
            ps = ps_c.tile([128, 260], f32, tag="ps_c", name=f"c{b}_{g}_{mb}")
            # each 65-col region is one accumulation chain; chains must not
            # interleave within a tile (start= resets has_written tracking)
            for hh in range(4):
                h = g * 4 + hh
                hp, h01 = h // 2, h % 2
                for nb in range(4):
                    nbh, nbl = nb // 2, nb % 2
                    u = u_tiles[(b, hp, nbh)]
                    nc.tensor.matmul(
                        ps[:, hh * 65: hh * 65 + 65],
                        u[:, h01 * 512 + nbl * 256 + mb * 128:
                          h01 * 512 + nbl * 256 + (mb + 1) * 128],
                        values[nb][:, h * 65:(h + 1) * 65],
                        start=(nb == 0), stop=(nb == 3),
                        skip_group_check=True,
                    )
            view = ps[:].rearrange("p (hh c) -> p hh c", c=65)
            # w_last/Z <= 5e-4, so the +w_last term is dropped (validated)
            rzi = rzpool.tile([128, 4], f32, tag="rzi", name=f"rzi{b}_{g}_{mb}")
            nc.vector.reciprocal(rzi[:], view[:, :, 64])
            nc.vector.tensor_tensor(
                out_sb[:, g * 256:(g + 1) * 256].rearrange(
                    "p (hh d) -> p hh d", d=64),
                view[:, :, 0:64],
                rzi[:][:, :, None].to_broadcast((128, 4, 64)),
                op=MULT,
            )

    # ---- pipelined emission ----
    # Ordering rules, derived from per-engine strict program order:
    #  * The PE stream must never place exp-gated work (ctx) ahead of
    #    DMA-gated work (b1 proj/scores) — the exps stream serially on the
    #    scalar engine and would stall the PE behind them.
    #  * The scalar engine does keysT-m0-b0 copy + all 16 exps, nothing
    #    else, so the exp stream runs bubble-free once started.
    #  * All other PSUM->SBUF copies and the normalizes go to vector,
    #    emitted in the order they are needed.
    emit_keys(0, 0, nc.scalar)
    emit_scores(0, 0)
    for m in range(1, 4):
        emit_keys(0, m, nc.vector)
        emit_scores(0, m)
    for m in range(4):
        emit_values(0, m)
    for et in (2, 0, 3, 1):
        emit_pool(1, et)
    for m in range(4):
        emit_keys(1, m, nc.vector)
    emit_scores(1, 0)
    for m in range(4):
        emit_values(1, m)
    emit_scores(1, 1)
    emit_ctx(0, 0)
    emit_scores(1, 2)
    emit_ctx(0, 1)
    for mb in range(2):
        nc.sync.dma_start(out_d[0, mb * 128:(mb + 1) * 128, :],
                          out_sbs[(0, mb)][:])
    emit_scores(1, 3)
    emit_ctx(1, 0)
    emit_ctx(1, 1)
    for mb in range(2):
        nc.scalar.dma_start(out_d[1, mb * 128:(mb + 1) * 128, :],
                            out_sbs[(1, mb)][:])


def _get_program(w_last):
    if "nc" not in _CACHE:
        _CACHE["nc"] = _build_program(w_last)
    return _CACHE["nc"]


def make_in_maps(k, q, Wk, Wv):
    import ml_dtypes
    wk, wv, wn, w_last = _host_constants(Wk, Wv)
    k16 = np.asarray(k).astype(ml_dtypes.bfloat16)
    # deinterleave l = 4n+j -> [b, e, j, n] so pooling is unit-stride adds
    k16 = np.ascontiguousarray(
        k16.reshape(B_FULL, E, N, 4).transpose(0, 1, 3, 2)).reshape(B_FULL, E, L)
    # qT packed to match SBUF layout [p, eb*256+t]: row p holds q^T rows
    # eb*128+p for eb=0..3 -> 2KB-contiguous DMA rows
    qT16 = np.asarray(q).astype(ml_dtypes.bfloat16).transpose(0, 2, 1)  # [B,E,T]
    qT16 = np.ascontiguousarray(
        qT16.reshape(B_FULL, 4, 128, T).transpose(0, 2, 1, 3).reshape(
            B_FULL, 128, 4 * T))
    # wk/wv packed to SBUF layout [p, kk*512+e'] (row e = kk*128+p)
    wk = np.ascontiguousarray(
        wk.reshape(4, 128, E).transpose(1, 0, 2).reshape(128, 4 * E))
    wv = np.ascontiguousarray(
        wv.reshape(4, 128, E).transpose(1, 0, 2).reshape(128, 4 * E))
    in_maps = []
    for c in range(N_CORES):
        in_maps.append({
            "k": np.ascontiguousarray(k16[c * B_PER:(c + 1) * B_PER]),
            "qT": np.ascontiguousarray(qT16[c * B_PER:(c + 1) * B_PER]),
            "wk": wk,
            "wv": wv,
            "wn": wn,
        })
    return in_maps, w_last


def kernel(k, q, Wk, Wv):
    from concourse.bass_utils import run_bass_kernel_spmd

    in_maps, w_last = make_in_maps(k, q, Wk, Wv)
    nc = _get_program(w_last)
    res = run_bass_kernel_spmd(nc, in_maps, core_ids=list(range(N_CORES)))
    out = np.concatenate([res.results[c]["out"] for c in range(N_CORES)], axis=0)
    return out.astype(np.float32)
